# revision 37
# baseline (speedup 1.0000x reference)
"""FEDformer-style DecoderLayer on 8 trn2 NeuronCores (Bass/Tile).

Sharding: data-parallel over batch (B=16 -> 2/core). FourierBlock mode
weights [D,D,64] are mode-sharded 8 ways. Each core DFTs its OWN 2 batches
(full 64 modes), an AllToAll redistributes spectra so core c holds all 16
batches for its 8 modes, mode matmuls run there, and a second AllToAll
returns per-mode spectra to the batch owner for the inverse DFT. All FFTs
are truncated DFT matmuls. FEB math in bf16 (weights have scale 1/D^2),
main GEMMs fp32r with ap>=256 (full PE rate), attention-logit path fp32.
"""
import sys
sys.path.insert(0, '/opt/trn_rl_repo')
import numpy as np
import ml_dtypes

import concourse.bass as bass
import concourse.bacc as bacc
import concourse.mybir as mybir
import concourse.tile as tile
from concourse.bass_utils import run_bass_kernel_spmd
from concourse.masks import make_identity

P = 128
B, L, S, D, H, M, DFF, CO = 16, 512, 1024, 512, 8, 64, 2048, 512
NC = 8
BL = B // NC            # 2 batches/core
MJ = M // NC            # 8 modes/core
DK = D // H             # 64
DT = D // P             # 4
LT = L // P             # 4
ST = S // P             # 8
FT = DFF // P           # 16

F32 = mybir.dt.float32
F32R = mybir.dt.float32r
F16 = mybir.dt.float16
BF16 = mybir.dt.bfloat16
AF = mybir.ActivationFunctionType
OP = mybir.AluOpType
AX = mybir.AxisListType

_CACHE = {}
DEBUG_TAPS = False


def _build():
    nc = bacc.Bacc("TRN2", target_bir_lowering=False, debug=False, num_devices=NC)

    def din(name, shape, dt=F32):
        return nc.dram_tensor(name, shape, dt, kind="ExternalInput")

    xtok = din("xtok", [BL, LT, P, D], BF16)     # own x token-major, bf16
    xfm = din("xfm", [BL, DT, P, L])             # own x feature-major
    crs = din("crs", [BL, ST, P, D], F32R)       # cross token-major chunks
    fwLb = din("fwLb", [LT, P, 2 * M], BF16)     # full fwd DFT basis (L)
    fwLf = din("fwLf", [LT, P, 2 * M], F32R)     # same basis, f32r (Q path)
    fw1024r = din("fw1024r", [ST, P, P], F32R)   # fwd DFT basis (S)
    iv512r = din("iv512r", [P, L], BF16)         # inverse DFT, A2A row order
    iv512b = din("iv512b", [P, L], F32R)         # inverse DFT, block rows
    febwr = din("febwr", [MJ, DT, P, D], BF16)
    febwi = din("febwi", [MJ, DT, P, D], BF16)
    wqT = din("wqT", [DT, P, D], F32R)
    wkT = din("wkT", [DT, P, D], F32R)
    wvT = din("wvT", [DT, P, D], F32R)
    woT = din("woT", [DT, P, D], F32R)
    dcb_kq = din("dcb_kq", [3, DT, P, 1])        # S*bk | L*bq | S*bv cols
    bo_pp = din("bo_pp", [DT, P, 1])
    wff1T = din("wff1T", [FT, DT, P, P], BF16)   # [ft][dc][p=d][ff-col]
    wff2T = din("wff2T", [FT, P, D], BF16)       # [fc][p=ff][e]
    wccT = din("wccT", [3, 3, DT, P, CO], F16)  # [trend][shift][dc][p=k][co]
    gw1T = din("gw1T", [3, DT, P, D // 2], BF16)
    gb1 = din("gb1", [3, 2, P, 1])
    gw2T = din("gw2T", [3, 2, P, 4], F32R)       # col 3 zero-pad
    grow = din("grow", [1, 16])                  # gb2 x3 (4 each) | kinv(4)
    sign_r = din("sign_r", [P, 1], BF16)

    xout = nc.dram_tensor("xout", [BL, DT, P, L], F32, kind="ExternalOutput")
    rtout = nc.dram_tensor("rtout", [3, BL, LT, P, CO], F32, kind="ExternalOutput")

    dbg = {}
    if DEBUG_TAPS:
        for nm, shp in (("x0", [P, BL, DT, L]), ("x1", [P, BL, DT, L]),
                        ("x2", [P, BL, DT, L]), ("x3", [P, BL, DT, L]),
                        ("x4", [P, BL, DT, L]), ("om", [BL, P, D]),
                        ("qftT", [P, DT, BL, 2 * M]),
                        ("kf", [P, BL, D]), ("qf", [P, BL, D]),
                        ("vfre", [DK, BL, D]), ("of", [P, BL, D]),
                        ("apre", [BL, P, DT, L]), ("tr1", [BL, P, DT, L]),
                        ("g1", [BL, 1, 3, L]), ("y", [BL, P, DT, D])):
            dbg[nm] = nc.dram_tensor("dbg_" + nm, shp, F32,
                                     kind="ExternalOutput")

    # spectra A2A: core n sends (own 2 batches, modes of dest, re/im, all d)
    cc_sp_in = nc.dram_tensor("cc_sp_in", [NC, 2, P, DT, BL, MJ], BF16)
    cc_sp_out = nc.dram_tensor("cc_sp_out", [NC, 2, P, DT, BL, MJ], BF16)
    # om A2A: core j-owner sends per-mode products back to batch owners
    cc_om_in = nc.dram_tensor("cc_om_in", [NC, 2, BL, 2, MJ, D], BF16)
    cc_om_out = nc.dram_tensor("cc_om_out", [NC, 2, BL, 2, MJ, D], BF16)

    ctxs = []

    with tile.TileContext(nc) as tc:
        def pool(name, bufs, space="SBUF"):
            cm = tc.tile_pool(name=name, bufs=bufs, space=space)
            p = cm.__enter__()
            ctxs.append(cm)
            return p

        cp = pool("cp", 1)
        act = pool("act", 1)
        fbw = pool("fbw", 6)         # FEB mode weights, 3 modes in flight
        xsp = pool("xsp", 2)         # rotating 2MB x-stage slots
        wr2 = pool("wr2", 2)         # rotating 1MB slots: wk,wv,wq,wo
        wk1 = pool("wk1", 1)         # single-buffered transients
        wk2 = pool("wk2", 2)         # double-buffered streams
        psA = pool("psA", 4, "PSUM")
        psC = pool("psC", 2, "PSUM")
        psB = pool("psB", 2, "PSUM")

        # ---------------- constants ----------------
        ident = cp.tile([P, P], F32, tag="ident")
        make_identity(nc, ident[:])
        warmid = psB.tile([P, P], F32, tag="psB")
        nc.tensor.transpose(warmid[:], ident[:], ident[:])

        fwLb_s = cp.tile([P, LT, 2 * M], BF16, tag="fwLb")
        nc.sync.dma_start(fwLb_s[:], fwLb.rearrange("c p m -> p c m"))
        fwLf_s = cp.tile([P, LT, 2 * M], F32R, tag="fwLf")
        nc.sync.dma_start(fwLf_s[:], fwLf.rearrange("c p m -> p c m"))
        fw1024_s = cp.tile([P, ST, P], F32R, tag="fw1024")
        nc.sync.dma_start(fw1024_s[:], fw1024r.rearrange("c p m -> p c m"))
        iv512_s = cp.tile([P, L], BF16, tag="iv512")
        nc.sync.dma_start(iv512_s[:], iv512r[:])
        iv512b_s = cp.tile([P, L], F32R, tag="iv512b")
        nc.sync.dma_start(iv512b_s[:], iv512b[:])
        sign_s = cp.tile([P, 1], BF16, tag="sign")
        nc.sync.dma_start(sign_s[:], sign_r[:])
        bo_s = cp.tile([P, DT, 1], F32, tag="bo")
        nc.sync.dma_start(bo_s[:], bo_pp.rearrange("c p o -> p c o"))
        gb1_s = cp.tile([P, 3, 2, 1], F32, tag="gb1")
        nc.sync.dma_start(gb1_s[:], gb1.rearrange("g h p o -> p g h o"))
        gw2_s = cp.tile([P, 3, 2, 4], F32R, tag="gw2")
        nc.sync.dma_start(gw2_s[:], gw2T.rearrange("g h p t -> p g h t"))
        dckq_s = cp.tile([P, 3, DT, 1], F32, tag="dckq")
        nc.sync.dma_start(dckq_s[:], dcb_kq.rearrange("k c p o -> p k c o"))
        grow_s = cp.tile([1, 16], F32, tag="grow")
        nc.sync.dma_start(grow_s[:], grow[:])
        gbc = cp.tile([P, 16], F32, tag="gbc")
        nc.gpsimd.partition_broadcast(gbc[:], grow_s[:])
        kinv_b = gbc[:, 12:15]



        febw_pre = []
        for j in range(2):
            wr_t = fbw.tile([P, DT, D], BF16, tag="febw")
            nc.gpsimd.dma_start(wr_t[:], febwr[j].rearrange("c p e -> p c e"))
            wi_t = fbw.tile([P, DT, D], BF16, tag="febw")
            nc.gpsimd.dma_start(wi_t[:], febwi[j].rearrange("c p e -> p c e"))
            febw_pre.append((wr_t, wi_t))

        # ============ S1: DFT own batches (full 64 modes) ===================
        qftT = act.tile([P, 2, NC, DT, BL, MJ], BF16, tag="qftT")
        for b in range(BL):
            xb = wk1.tile([P, LT, D], BF16, tag="xtok_b")
            nc.scalar.dma_start(xb[:], xtok[b].rearrange("t p d -> p t d"))
            for dc in range(DT):
                pd = psB.tile([P, P], F32, tag="psB")
                for lc in range(LT):
                    nc.tensor.matmul(pd[:, 0:2 * M],
                                     xb[:, lc, dc * P:(dc + 1) * P],
                                     fwLb_s[:, lc, :],
                                     start=(lc == 0), stop=(lc == LT - 1))
                nc.vector.tensor_copy(
                    qftT[:, :, :, dc, b, :],
                    pd[:, 0:2 * M].rearrange("p (r n j) -> p r n j",
                                             r=2, n=NC, j=MJ))

        # scatter spectra to mode owners
        for n in range(NC):
            for ri in range(2):
                nc.sync.dma_start(cc_sp_in[n, ri], qftT[:, ri, n])
        nc.gpsimd.collective_compute(
            "AllToAll", OP.bypass, replica_groups=[list(range(NC))],
            ins=[cc_sp_in[:]], outs=[cc_sp_out[:]])

        # ============ C1: cross DFT (overlaps spectra A2A) ==================
        crossFd = act.tile([P, DT, BL, P], F32R, tag="crossFd")
        for b in range(BL):
            pm = psA.tile([P, 512], F32, tag="psA")
            for sc in range(ST):
                cx = wk2.tile([P, D], F32R, tag="crs_c")
                nc.scalar.dma_start(cx[:], crs[b, sc])
                nc.tensor.matmul(pm[:], fw1024_s[:, sc, :], cx[:],
                                 start=(sc == 0), stop=(sc == ST - 1))
            cF = wk1.tile([P, 512], F32, tag="cF")
            nc.vector.tensor_copy(cF[:], pm[:])
            for dc in range(DT):
                pt = psB.tile([P, P], F32, tag="psB")
                nc.tensor.transpose(pt[:], cF[:, dc * P:(dc + 1) * P],
                                    ident[:])
                nc.vector.tensor_copy(crossFd[:, dc, b, :], pt[:])

        # ============ K/V proj in mode space ================================
        wk_s = wr2.tile([P, DT, D], F32R, tag="wr2")
        nc.scalar.dma_start(wk_s[:], wkT.rearrange("c p e -> p c e"))
        wv_s = wr2.tile([P, DT, D], F32R, tag="wr2")
        nc.scalar.dma_start(wv_s[:], wvT.rearrange("c p e -> p c e"))

        vf_re = act.tile([DK, BL, D], F32, tag="vf_re")
        vf_im = act.tile([DK, BL, D], F32, tag="vf_im")
        kf_d = act.tile([P, BL, DT, P], F32, tag="kf_d")
        qf_d = act.tile([P, BL, DT, P], F32, tag="qf_d")

        for wmat, kq, dest in ((wk_s, 0, kf_d), (wv_s, 2, None)):
            for et in range(DT):
                pk = psC.tile([P, 2 * P], F32, tag="psC")
                for dc in range(DT):
                    nc.tensor.matmul(
                        pk[:], wmat[:, dc, et * P:(et + 1) * P],
                        crossFd[:, dc].rearrange("p b m -> p (b m)"),
                        start=(dc == 0), stop=(dc == DT - 1))
                if dest is not None:
                    for b in range(BL):
                        tgt = dest[:, b, et, :]
                        nc.scalar.copy(tgt, pk[:, b * P:(b + 1) * P])
                        nc.vector.tensor_add(tgt[:, 0:1], tgt[:, 0:1],
                                             dckq_s[:, kq, et, :])
                else:
                    vtmp = wk1.tile([P, BL, P], F32, tag="vtmp")
                    nc.scalar.copy(vtmp[:], pk[:])
                    for b in range(BL):
                        nc.vector.tensor_add(vtmp[:, b, 0:1], vtmp[:, b, 0:1],
                                             dckq_s[:, kq, et, :])
                        ptr = psB.tile([DK, P], F32, tag="psB")
                        nc.tensor.transpose(ptr[:], vtmp[:, b, 0:DK],
                                            ident[:])
                        nc.vector.tensor_copy(
                            vf_re[:, b, et * P:(et + 1) * P], ptr[:])
                        pti = psB.tile([DK, P], F32, tag="psB")
                        nc.tensor.transpose(pti[:], vtmp[:, b, DK:P],
                                            ident[:])
                        nc.vector.tensor_copy(
                            vf_im[:, b, et * P:(et + 1) * P], pti[:])

        # ============ A2: per-mode matmuls on gathered spectra ==============
        qA = act.tile([P, DT, 2 * BL * NC, MJ], BF16, tag="qA")
        for n in range(NC):
            for ri in range(2):
                c0 = n * 2 * BL + ri * BL
                nc.sync.dma_start(qA[:, :, c0:c0 + BL, :],
                                  cc_sp_out[n, ri])
        for j in range(MJ):
            if j < 2:
                wr_t, wi_t = febw_pre[j]
            else:
                wr_t = fbw.tile([P, DT, D], BF16, tag="febw")
                nc.gpsimd.dma_start(wr_t[:],
                                    febwr[j].rearrange("c p e -> p c e"))
                wi_t = fbw.tile([P, DT, D], BF16, tag="febw")
                nc.gpsimd.dma_start(wi_t[:],
                                    febwi[j].rearrange("c p e -> p c e"))
            g1 = psA.tile([32, 512], F32, tag="psA")
            g2 = psA.tile([32, 512], F32, tag="psA")
            for dc in range(DT):
                lh = qA[:, dc, :, j]
                nc.tensor.matmul(g1[:], lh, wr_t[:, dc, :],
                                 start=(dc == 0), stop=(dc == DT - 1))
                nc.tensor.matmul(g2[:], lh, wi_t[:, dc, :],
                                 start=(dc == 0), stop=(dc == DT - 1))
            sg = wk1.tile([32, 2, 512], BF16, tag="stg")
            nc.vector.tensor_copy(sg[:, 0, :], g1[:])
            nc.vector.tensor_copy(sg[:, 1, :], g2[:])
            nc.sync.dma_start(cc_om_in[:, :, :, 0, j, :], sg[:, 0, :])
            nc.sync.dma_start(cc_om_in[:, :, :, 1, j, :], sg[:, 1, :])

        nc.gpsimd.collective_compute(
            "AllToAll", OP.bypass, replica_groups=[list(range(NC))],
            ins=[cc_om_in[:]], outs=[cc_om_out[:]])

        # ============ A4: om assembly, IDFT, FEB residual ===================
        xfm_s = xsp.tile([P, BL, DT, L], F32, tag="xs")
        nc.scalar.dma_start(xfm_s[:], xfm.rearrange("b c p l -> p b c l"))
        x0 = xsp.tile([P, BL, DT, L], F32, tag="xs")
        for b in range(BL):
            t1 = wk1.tile([P, D], BF16, tag="a2a")
            t2 = wk1.tile([P, D], BF16, tag="a2b")
            nc.sync.dma_start(t1[:], cc_om_out[:, :, b, 0, :, :])
            for n in range(NC):
                nc.sync.dma_start(t2[n * 16:n * 16 + 8],
                                  cc_om_out[n, 1, b, 1])
                nc.sync.dma_start(t2[n * 16 + 8:n * 16 + 16],
                                  cc_om_out[n, 0, b, 1])
            om_t = wk1.tile([P, BL, D], BF16, tag="om", name="om_t")
            om = om_t[:, 0, :]
            nc.vector.scalar_tensor_tensor(om[:], t2[:], sign_s[:], t1[:],
                                           op0=OP.mult, op1=OP.add)
            if DEBUG_TAPS:
                omf = wk1.tile([P, D], F32, tag="omf")
                nc.vector.tensor_copy(omf[:], om[:])
                nc.sync.dma_start(dbg["om"][b], omf[:])
            for et in range(DT):
                pi = psA.tile([P, 512], F32, tag="psA")
                nc.tensor.matmul(pi[:], om[:, et * P:(et + 1) * P], iv512_s[:],
                                 start=True, stop=True)
                nc.vector.tensor_add(x0[:, b, et, :], xfm_s[:, b, et, :],
                                     pi[:])

        if DEBUG_TAPS:
            nc.sync.dma_start(dbg["x0"][:], x0[:].bitcast(F32))
            nc.sync.dma_start(dbg["vfre"][:], vf_re[:])
            nc.sync.dma_start(dbg["kf"][:], kf_d[:])

        # ============ shared decomposition block ============================
        def decomp(xin, xout_t, widx, after_b=None):
            gw1_w = wk1.tile([P, DT, D // 2], BF16, tag="gwst")
            nc.scalar.dma_start(gw1_w[:], gw1T[widx].rearrange("c p h -> p c h"))
            gb2_b = gbc[:, widx * 4:widx * 4 + 3]
            gbts = []
            for b in range(BL):
                xbf = wk1.tile([P, DT, L], BF16, tag="xbf")
                nc.scalar.copy(xbf[:], xin[:, b])
                h = wk1.tile([P, 2, L], F32R, tag="g_h")
                for ht in range(2):
                    ph = psC.tile([P, 512], F32, tag="psC")
                    for dc in range(DT):
                        nc.tensor.matmul(ph[:],
                                         gw1_w[:, dc, ht * P:(ht + 1) * P],
                                         xbf[:, dc, :],
                                         start=(dc == 0), stop=(dc == DT - 1))
                    nc.scalar.activation(h[:, ht, :], ph[:], AF.Relu,
                                         bias=gb1_s[:, widx, ht, :], scale=1.0)
                pg = psB.tile([P, LT, 4], F32, tag="psB")
                for lt_i in range(LT):
                    for hc in range(2):
                        nc.tensor.matmul(pg[:, lt_i, :],
                                         h[:, hc, lt_i * P:(lt_i + 1) * P],
                                         gw2_s[:, widx, hc, :],
                                         start=(hc == 0), stop=(hc == 1),
                                         skip_group_check=True)
                gt4 = wk1.tile([P, LT, 4], F32, tag="g_t")
                nc.vector.tensor_add(
                    gt4[:, :, 0:3], pg[:, :, 0:3],
                    gb2_b.unsqueeze(1).broadcast_to([P, LT, 3]))
                mx4 = wk1.tile([P, LT], F32, tag="g_mx")
                nc.vector.tensor_reduce(mx4[:], gt4[:, :, 0:3], axis=AX.X,
                                        op=OP.max, negate=True)
                nc.vector.tensor_add(
                    gt4[:, :, 0:3], gt4[:, :, 0:3],
                    mx4[:].unsqueeze(2).broadcast_to([P, LT, 3]))
                nc.scalar.activation(gt4[:, :, 0:3], gt4[:, :, 0:3], AF.Exp)
                sm4 = wk1.tile([P, LT], F32, tag="g_sm")
                nc.vector.tensor_reduce(sm4[:], gt4[:, :, 0:3], axis=AX.X,
                                        op=OP.add)
                rc4 = wk1.tile([P, LT], F32, tag="g_rc")
                nc.vector.reciprocal(rc4[:], sm4[:])
                nc.vector.tensor_mul(
                    gt4[:, :, 0:3], gt4[:, :, 0:3],
                    rc4[:].unsqueeze(2).broadcast_to([P, LT, 3]))
                nc.vector.tensor_mul(
                    gt4[:, :, 0:3], gt4[:, :, 0:3],
                    kinv_b.unsqueeze(1).broadcast_to([P, LT, 3]))
                nc.vector.tensor_add(gt4[:, :, 1:2], gt4[:, :, 1:2],
                                     gt4[:, :, 2:3])
                nc.vector.tensor_add(gt4[:, :, 0:1], gt4[:, :, 0:1],
                                     gt4[:, :, 1:2])
                gfm = wk1.tile([1, 3, L], F16, tag="stg")
                for lt_i in range(LT):
                    for e in range(3):
                        pgt = psB.tile([1, P], F32, tag="psB")
                        nc.tensor.transpose(pgt[:],
                                            gt4[:, lt_i, e:e + 1], ident[:])
                        nc.vector.tensor_copy(
                            gfm[:, e, lt_i * P:(lt_i + 1) * P], pgt[:])
                gbt = wk2.tile([P, 3, L], F16, tag="g_gb")
                for e in range(3):
                    nc.gpsimd.partition_broadcast(gbt[:, e, :], gfm[:, e, :])
                if DEBUG_TAPS and widx == 0:
                    gfmf = wk1.tile([1, 3, L], F32, tag="stgf")
                    nc.vector.tensor_copy(gfmf[:], gfm[:])
                    nc.sync.dma_start(dbg["g1"][b], gfmf[:])
                gbts.append(gbt)
            for b in range(BL):
                trend_b = wk2.tile([P, DT, L + 2], F16, tag="trend")
                gbt = gbts[b]
                for dt_i in range(DT):
                    eng = nc.vector
                    sfx = "v"
                    pad = wk1.tile([P, L + 6], F16, tag="d_pad" + sfx)
                    nc.gpsimd.memset(pad[:, 0:3], 0.0)
                    nc.gpsimd.memset(pad[:, L + 3:L + 6], 0.0)
                    nc.scalar.copy(pad[:, 3:L + 3], xin[:, b, dt_i, :])
                    sb = wk1.tile([P, L], F16, tag="d_s" + sfx)
                    tmp = wk1.tile([P, L], F16, tag="d_tmp" + sfx)
                    acc = wk1.tile([P, L], F16, tag="d_acc" + sfx)
                    eng.tensor_add(sb[:], pad[:, 2:L + 2], pad[:, 3:L + 3])
                    eng.tensor_add(sb[:], sb[:], pad[:, 4:L + 4])
                    eng.tensor_mul(acc[:], sb[:], gbt[:, 0, :])
                    eng.tensor_add(tmp[:], pad[:, 1:L + 1], pad[:, 5:L + 5])
                    eng.tensor_mul(tmp[:], tmp[:], gbt[:, 1, :])
                    eng.tensor_add(acc[:], acc[:], tmp[:])
                    eng.tensor_add(sb[:], pad[:, 0:L], pad[:, 6:L + 6])
                    eng.tensor_mul(sb[:], sb[:], gbt[:, 2, :])
                    eng.tensor_add(acc[:], acc[:], sb[:])
                    nc.scalar.copy(trend_b[:, dt_i, 1:L + 1], acc[:])
                    eng.tensor_sub(xout_t[:, b, dt_i, :],
                                   xin[:, b, dt_i, :], acc[:])
                if DEBUG_TAPS and widx == 0:
                    for _dc in range(DT):
                        trf = wk1.tile([P, L], F32, tag="d_tmpv")
                        nc.vector.tensor_copy(trf[:],
                                              trend_b[:, _dc, 1:L + 1])
                        nc.sync.dma_start(dbg["tr1"][b, :, _dc], trf[:])
                circpass_b(trend_b, widx, b)
                if after_b is not None:
                    after_b(b)

        # ============ circ-conv partial pass (per batch) ====================
        def circpass_b(trend_b, widx, b):
            nc.gpsimd.tensor_copy(trend_b[:, :, 0:1], trend_b[:, :, L:L + 1])
            nc.gpsimd.tensor_copy(trend_b[:, :, L + 1:L + 2],
                                  trend_b[:, :, 1:2])
            prs = [psA.tile([P, 512], F32, tag="psA", name=f"pcc{_i}")
                   for _i in range(LT)]
            for s in range(3):
                wcc_w = wk2.tile([P, DT, CO], F16, tag="wcc")
                nc.scalar.dma_start(wcc_w[:],
                                    wccT[widx, s].rearrange("c p o -> p c o"))
                for lt_i in range(LT):
                    for dc in range(DT):
                        nc.tensor.matmul(
                            prs[lt_i][:],
                            trend_b[:, dc, lt_i * P + s:lt_i * P + s + P],
                            wcc_w[:, dc, :],
                            start=(s == 0 and dc == 0),
                            stop=(s == 2 and dc == DT - 1),
                            skip_group_check=True)
            for lt_i in range(LT):
                rst = wk1.tile([P, CO], F32, tag="rtst")
                nc.vector.tensor_copy(rst[:], prs[lt_i][:])
                nc.sync.dma_start(rtout[widx, b, lt_i], rst[:])

        wq_s = wr2.tile([P, DT, D], F32R, tag="wr2")
        nc.scalar.dma_start(wq_s[:], wqT.rearrange("c p e -> p c e"))
        x1 = xsp.tile([P, BL, DT, L], F32, tag="xs")

        def qproj_b(b):
            x1b = wk1.tile([P, DT, L], F32R, tag="m8k")
            nc.scalar.copy(x1b[:], x1[:, b])
            pqf = [psA.tile([P, P], F32, tag="psA", name=f"pqf{_i}")
                   for _i in range(DT)]
            for lc in range(LT):
                pk = psC.tile([P, 512], F32, tag="psC")
                for dc in range(DT):
                    nc.tensor.matmul(pk[:],
                                     x1b[:, dc, lc * P:(lc + 1) * P],
                                     wq_s[:, dc, :],
                                     start=(dc == 0), stop=(dc == DT - 1))
                qt = wk2.tile([P, D], F32R, tag="kv_tt")
                nc.scalar.copy(qt[:], pk[:])
                for dt_i in range(DT):
                    nc.tensor.matmul(pqf[dt_i][:],
                                     qt[:, dt_i * P:(dt_i + 1) * P],
                                     fwLf_s[:, lc, :],
                                     start=(lc == 0), stop=(lc == LT - 1),
                                     skip_group_check=True)
            for dt_i in range(DT):
                nc.scalar.copy(qf_d[:, b, dt_i, :], pqf[dt_i][:])
                nc.vector.tensor_add(qf_d[:, b, dt_i, 0:1],
                                     qf_d[:, b, dt_i, 0:1],
                                     dckq_s[:, 1, dt_i, :])

        decomp(x0, x1, 0, after_b=qproj_b)
        if DEBUG_TAPS:
            nc.sync.dma_start(dbg["x1"][:], x1[:].bitcast(F32))

        if DEBUG_TAPS:
            nc.sync.dma_start(dbg["qf"][:], qf_d[:])

        # ============ attention =============================================
        of_sb = wk1.tile([P, BL, D], F32R, tag="om")
        for b in range(BL):
            sall = wk1.tile([DK, H, M], F32, tag="s_all")
            for hh in range(H):
                blk, half = hh // 2, (hh % 2) * DK
                pS = psB.tile([DK, M], F32, tag="psB")
                for ri in range(2):
                    nc.tensor.matmul(
                        pS[:],
                        qf_d[half:half + DK, b, blk, ri * M:(ri + 1) * M],
                        kf_d[half:half + DK, b, blk, ri * M:(ri + 1) * M],
                        start=(ri == 0), stop=(ri == 1))
                nc.vector.tensor_copy(sall[:, hh, :], pS[:])
            mx = wk1.tile([DK, H], F32, tag="s_mx")
            nc.vector.tensor_reduce(mx[:], sall[:], axis=AX.X, op=OP.max,
                                    negate=True)
            nc.vector.tensor_add(
                sall[:], sall[:],
                mx[:].unsqueeze(2).broadcast_to([DK, H, M]))
            nc.scalar.activation(sall[:], sall[:], AF.Exp)
            sm = wk1.tile([DK, H], F32, tag="s_sm")
            nc.vector.tensor_reduce(sm[:], sall[:], axis=AX.X, op=OP.add)
            rc = wk1.tile([DK, H], F32, tag="s_rc")
            nc.vector.reciprocal(rc[:], sm[:])
            nc.vector.tensor_mul(
                sall[:], sall[:],
                rc[:].unsqueeze(2).broadcast_to([DK, H, M]))
            aT = wk1.tile([DK, H, M], F32, tag="a_T")
            for hh in range(H):
                pt = psB.tile([DK, M], F32, tag="psB")
                nc.tensor.transpose(pt[:], sall[:, hh, :], ident[0:DK, 0:DK])
                nc.vector.tensor_copy(aT[:, hh, :], pt[:])
            pof = psA.tile([P, 512], F32, tag="psA")
            for hh in range(H):
                nc.tensor.matmul(pof[0:DK, hh * DK:(hh + 1) * DK],
                                 aT[:, hh, :],
                                 vf_re[:, b, hh * DK:(hh + 1) * DK],
                                 start=True, stop=True)
                nc.tensor.matmul(pof[DK:P, hh * DK:(hh + 1) * DK],
                                 aT[:, hh, :],
                                 vf_im[:, b, hh * DK:(hh + 1) * DK],
                                 start=True, stop=True)
            nc.vector.tensor_copy(of_sb[:, b, :], pof[:])

        if DEBUG_TAPS:
            nc.sync.dma_start(dbg["of"][:], of_sb[:].bitcast(F32))

        # idft (fm) -> wo proj + bias + residual -> x2
        wo_s = wr2.tile([P, DT, D], F32R, tag="wr2")
        nc.scalar.dma_start(wo_s[:], woT.rearrange("c p e -> p c e"))
        x2 = xsp.tile([P, BL, DT, L], F32, tag="xs")
        for b in range(BL):
            apre = wk1.tile([P, DT, L], F32R, tag="m8k")
            for et in range(DT):
                pi = psA.tile([P, 512], F32, tag="psA")
                nc.tensor.matmul(pi[:], of_sb[:, b, et * P:(et + 1) * P],
                                 iv512b_s[:], start=True, stop=True)
                nc.scalar.copy(apre[:, et, :], pi[:])
            if DEBUG_TAPS:
                nc.sync.dma_start(dbg["apre"][b], apre[:].bitcast(F32))
            for et in range(DT):
                po = psA.tile([P, 512], F32, tag="psA")
                for dc in range(DT):
                    nc.tensor.matmul(po[:], wo_s[:, dc, et * P:(et + 1) * P],
                                     apre[:, dc, :],
                                     start=(dc == 0), stop=(dc == DT - 1))
                nc.vector.scalar_tensor_tensor(
                    x2[:, b, et, :], po[:], bo_s[:, et, :],
                    x1[:, b, et, :], op0=OP.add, op1=OP.add)

        # ============ decomp2 / FFN / decomp3 ===============================
        if DEBUG_TAPS:
            nc.sync.dma_start(dbg["x2"][:], x2[:].bitcast(F32))
        x3 = xsp.tile([P, BL, DT, L], F32, tag="xs")
        x4 = xsp.tile([P, BL, DT, L], F32, tag="xs")

        def ffn_b(b):
            x3b = wk1.tile([P, DT, L], BF16, tag="xbf")
            nc.scalar.copy(x3b[:], x3[:, b])
            y_sb = wk1.tile([P, DT, D], F32, tag="m8k")
            for f in range(4):
                h = wk1.tile([P, FT // 4, L], BF16, tag="ffn_h")
                for fi in range(FT // 4):
                    ft = f * (FT // 4) + fi
                    w1 = wk2.tile([P, DT, P], BF16, tag="wst")
                    nc.scalar.dma_start(w1[:],
                                        wff1T[ft].rearrange("c p o -> p c o"))
                    ph = psC.tile([P, 512], F32, tag="psC")
                    for dc in range(DT):
                        nc.tensor.matmul(ph[:], w1[:, dc, :],
                                         x3b[:, dc, :],
                                         start=(dc == 0), stop=(dc == DT - 1))
                    nc.scalar.activation(h[:, fi, :], ph[:], AF.Relu)
                pys = [psA.tile([P, 512], F32, tag="psA", name=f"py{_i}")
                       for _i in range(DT)]
                for fi in range(FT // 4):
                    fc = f * (FT // 4) + fi
                    w2 = wk2.tile([P, D], BF16, tag="wst2")
                    nc.scalar.dma_start(w2[:], wff2T[fc])
                    for et in range(DT):
                        nc.tensor.matmul(pys[et][:],
                                         w2[:, et * P:(et + 1) * P],
                                         h[:, fi, :],
                                         start=(fi == 0),
                                         stop=(fi == FT // 4 - 1))
                for et in range(DT):
                    if f == 0:
                        nc.vector.tensor_copy(y_sb[:, et, :], pys[et][:])
                    else:
                        nc.vector.tensor_add(y_sb[:, et, :], y_sb[:, et, :],
                                             pys[et][:])
            for et in range(DT):
                nc.vector.tensor_add(x4[:, b, et, :],
                                     x3[:, b, et, :],
                                     y_sb[:, et, :])
            if DEBUG_TAPS:
                nc.sync.dma_start(dbg["y"][b], y_sb[:])

        decomp(x2, x3, 1, after_b=ffn_b)
        if DEBUG_TAPS:
            nc.sync.dma_start(dbg["x3"][:], x3[:].bitcast(F32))
            nc.sync.dma_start(dbg["x4"][:], x4[:].bitcast(F32))
        x5 = xsp.tile([P, BL, DT, L], F32, tag="xs")

        def out_b(b):
            nc.sync.dma_start(xout[b].rearrange("c p l -> p c l"), x5[:, b])

        decomp(x4, x5, 2, after_b=out_b)

        for cm in reversed(ctxs):
            cm.__exit__(None, None, None)

    nc.compile()
    return nc


# ---------------------------------------------------------------------------
# host side
# ---------------------------------------------------------------------------
def _fwd_basis_cols(n, modes):
    l = np.arange(n)[:, None].astype(np.float64)
    m = np.asarray(modes)[None, :].astype(np.float64)
    th = 2.0 * np.pi * l * m / n
    return np.concatenate([np.cos(th), -np.sin(th)], axis=1).astype(np.float32)


def _inv_basis(n):
    l = np.arange(n)[None, :].astype(np.float64)
    m = np.arange(M)[:, None].astype(np.float64)
    c = np.where(np.arange(M) == 0, 1.0, 2.0)[:, None]
    th = 2.0 * np.pi * l * m / n
    return np.concatenate([c * np.cos(th) / n, -c * np.sin(th) / n],
                         axis=0).astype(np.float32)


def _prep_in_maps(x, cross, feb_wr, feb_wi, wq, bq, wk, bk, wv, bv, wo, bo,
                  w_ff1, w_ff2, d1_w1, d1_b1, d1_w2, d1_b2,
                  d2_w1, d2_b1, d2_w2, d2_b2, d3_w1, d3_b1, d3_w2, d3_b2,
                  p1, p2, p3):
    bf16 = ml_dtypes.bfloat16
    x = np.ascontiguousarray(x, np.float32)
    cross = np.ascontiguousarray(cross, np.float32)

    xtok_full = np.ascontiguousarray(x.reshape(B, LT, P, D).astype(bf16))
    xfm_full = np.ascontiguousarray(x.transpose(0, 2, 1)).reshape(B, DT, P, L)
    crs_full = np.ascontiguousarray(cross.reshape(B, ST, P, D))

    fwL_f32 = _fwd_basis_cols(L, np.arange(M))
    fwLb_np = np.ascontiguousarray(fwL_f32.astype(bf16).reshape(LT, P, 2 * M))
    fwLf32_np = np.ascontiguousarray(fwL_f32.reshape(LT, P, 2 * M))
    fw1024r_np = np.ascontiguousarray(
        _fwd_basis_cols(S, np.arange(M)).reshape(ST, P, P))
    iv512b_np = _inv_basis(L)
    iv512_np = _inv_basis(L)
    # om rows arrive as (src_core n, ri, local mode j): row n*16+ri*8+j holds
    # (re if ri==0 else im) of global mode n*8+j
    perm = np.zeros(P, np.int64)
    for n_i in range(NC):
        for ri in range(2):
            for j_i in range(MJ):
                perm[n_i * 16 + ri * 8 + j_i] = ri * M + n_i * MJ + j_i
    iv512_np = np.ascontiguousarray(iv512_np[perm]).astype(bf16)

    wqT_np = np.ascontiguousarray(wq.T).reshape(DT, P, D)
    wkT_np = np.ascontiguousarray(wk.T).reshape(DT, P, D)
    wvT_np = np.ascontiguousarray(wv.T).reshape(DT, P, D)
    woT_np = np.ascontiguousarray(wo.T).reshape(DT, P, D)
    dcb_kq_np = np.stack([np.asarray(bk) * S, np.asarray(bq) * L,
                          np.asarray(bv) * S]) \
        .reshape(3, DT, P, 1).astype(np.float32)
    bo_np = np.ascontiguousarray(bo).reshape(DT, P, 1).astype(np.float32)
    wff1_np = np.ascontiguousarray(
        w_ff1.T.reshape(DT, P, FT, P).transpose(2, 0, 1, 3)).astype(bf16)
    wff2_np = np.ascontiguousarray(w_ff2.T).reshape(FT, P, D).astype(bf16)
    wcc_np = np.zeros((3, 3, DT, P, CO), np.float16)
    for w_i, p_i in enumerate((p1, p2, p3)):
        for s in range(3):
            wcc_np[w_i, s] = np.ascontiguousarray(p_i[:, :, s].T) \
                .reshape(DT, P, CO)
    gw1_np = np.stack([np.ascontiguousarray(w.T).reshape(DT, P, D // 2)
                       for w in (d1_w1, d2_w1, d3_w1)]).astype(bf16)
    gb1_np = np.stack([np.asarray(b_).reshape(2, P, 1)
                       for b_ in (d1_b1, d2_b1, d3_b1)]).astype(np.float32)
    gw2_np = np.zeros((3, 2, P, 4), np.float32)
    for i, w in enumerate((d1_w2, d2_w2, d3_w2)):
        gw2_np[i, :, :, 0:3] = np.ascontiguousarray(w.T).reshape(2, P, 3)
    grow_np = np.zeros((1, 16), np.float32)
    for i, b2 in enumerate((d1_b2, d2_b2, d3_b2)):
        grow_np[0, i * 4:i * 4 + 3] = np.asarray(b2, np.float32)
    grow_np[0, 12:15] = [1.0 / 3.0, 1.0 / 5.0, 1.0 / 7.0]
    sign_np = np.tile(np.concatenate([-np.ones(8), np.ones(8)]), NC) \
        .reshape(P, 1).astype(bf16)

    in_maps = []
    for c in range(NC):
        bs = slice(BL * c, BL * (c + 1))
        in_maps.append(dict(
            xtok=xtok_full[bs],
            xfm=xfm_full[bs],
            crs=crs_full[bs],
            fwLb=fwLb_np, fwLf=fwLf32_np,
            fw1024r=fw1024r_np, iv512r=iv512_np,
            iv512b=iv512b_np,
            febwr=np.ascontiguousarray(
                feb_wr[:, :, MJ * c:MJ * (c + 1)].transpose(2, 0, 1))
                .astype(bf16).reshape(MJ, DT, P, D),
            febwi=np.ascontiguousarray(
                feb_wi[:, :, MJ * c:MJ * (c + 1)].transpose(2, 0, 1))
                .astype(bf16).reshape(MJ, DT, P, D),
            wqT=wqT_np, wkT=wkT_np, wvT=wvT_np, woT=woT_np,
            dcb_kq=dcb_kq_np, bo_pp=bo_np,
            wff1T=wff1_np, wff2T=wff2_np, wccT=wcc_np,
            gw1T=gw1_np, gb1=gb1_np, gw2T=gw2_np,
            grow=grow_np, sign_r=sign_np,
        ))

    return in_maps


def kernel(**inputs):
    if "nc" not in _CACHE:
        _CACHE["nc"] = _build()
    nc = _CACHE["nc"]
    in_maps = _prep_in_maps(**inputs)
    _CACHE["in_maps"] = in_maps
    res = run_bass_kernel_spmd(nc, in_maps, list(range(NC)))
    xo = np.zeros((B, L, D), np.float32)
    rt = np.zeros((B, L, CO), np.float32)
    for c in range(NC):
        r = res.results[c]
        xo[BL * c:BL * (c + 1)] = np.asarray(r["xout"]) \
            .reshape(BL, D, L).transpose(0, 2, 1)
        rt[BL * c:BL * (c + 1)] = np.asarray(r["rtout"]) \
            .reshape(3, BL, L, CO).sum(axis=0)
    return xo, rt
